# revision 19
# baseline (speedup 1.0000x reference)
"""ChannelFC Trainium2 kernel: per-feature Linear y[b,f,:] = x[b,f,:] @ W[f].T + bias[f].

Shapes: x [64, 64, 32, 32], weight [64, 1024, 1024], bias [64, 1024].
Strategy: feature-parallel over 8 NeuronCores (8 features/core), fp16 operands
(fp32 PSUM accumulation), X-stationary matmuls (lhsT = X_f^T k-tiles [128, 64],
rhs = W_f^T k-tiles [128, 512]), bias added via a K=1 ones-vector matmul into
the same PSUM accumulation group. Weight stream is the roofline: 16 MB/core fp16.
"""

import numpy as np

import concourse.bass as bass
import concourse.mybir as mybir
from concourse.tile import TileContext
from concourse.vector_clock import ScopedClock


def _install_lean_tail_patch():
    """Tile's exit sequence is drain -> barrier -> sem-clear -> barrier
    (~7us measured). The final barrier only guards engines re-entering the
    sem space after the clear; at NEFF end nothing follows, and the next
    execution starts only after every engine's stream (including the
    GpSimd clear) has completed. Dropping it saves ~3-4us per run."""
    if getattr(TileContext, "_lean_tail", False):
        return

    def _drain_and_barrier(self, tick_clock, wait_clock):
        drain_inst = self.nc.sync.drain()
        wait_clock.add_sem_waits(
            drain_inst.ins, ScopedClock({None: tick_clock.global_clock})
        )
        self.nc.all_engine_barrier()
        assert self.sems is not None
        popped = self.nc._tile_sem_poison_stack.pop()
        assert popped is self._sem_poison
        self.nc.clear_and_free_semaphores(list(self.sems.allocated().values()))

    TileContext._drain_and_barrier = _drain_and_barrier
    TileContext._lean_tail = True


_install_lean_tail_patch()

B, F, C = 64, 64, 1024
NCORES = 8
FPC = F // NCORES  # features per core
KT = C // 128  # k-tiles of 128
NT = 2  # n-tiles of 512 (PSUM bank limit)
WH = 2  # W halves per feature (pipeline granularity)
KH = KT // WH  # k-tiles per half

_FP16 = mybir.dt.float16
_FP32 = mybir.dt.float32


def _split_sync_waits(nc, maxw=1):
    """This container's walrus build rejects more than one sync wait on an
    instruction ("Too many sync wait commands" in codegen). Hoist extra waits
    into same-engine NOPs placed immediately before the instruction —
    semantically identical since the engine sequencer blocks on each in order."""
    n = 0
    for fn in nc.m.functions:
        for bb in fn.blocks:
            new = []
            for inst in bb.instructions:
                si = getattr(inst, "sync_info", None)
                waits = list(si.on_wait or []) if si is not None else []
                if len(waits) > maxw:
                    extra, keep = waits[:-maxw], waits[-maxw:]
                    for i in range(0, len(extra), maxw):
                        n += 1
                        new.append(
                            mybir.InstNoOp(
                                name=f"WSPLIT-{n}",
                                engine=inst.engine,
                                bass_nofuse=True,
                                sync_info=mybir.SyncInfo(
                                    on_wait=extra[i : i + maxw], on_update=[]
                                ),
                            )
                        )
                    inst.sync_info = mybir.SyncInfo(
                        on_wait=keep, on_update=list(si.on_update or [])
                    )
                new.append(inst)
            bb.instructions = new


N_WARM = 22  # dummy K=1 N=512 matmuls bridging PE from t~8us to W0h0 (~13.5us)
# so the HAM clock gate is warm when real matmuls start.


def _build_program():
    nc = bass.Bass()
    xt = nc.dram_tensor("xt", [128, FPC, KT, B], _FP16, kind="ExternalInput")
    wt = nc.dram_tensor("wt", [FPC, 128, KT, C], _FP16, kind="ExternalInput")
    bs = nc.dram_tensor("bs", [FPC, 1, C], _FP16, kind="ExternalInput")
    y = nc.dram_tensor("y", [FPC, B, C], _FP16, kind="ExternalOutput")

    with TileContext(nc) as tc:
        with (
            tc.tile_pool(name="wpool", bufs=2 * FPC) as wpool,
            tc.tile_pool(name="const", bufs=1) as const_pool,
            tc.tile_pool(name="opool", bufs=3) as opool,
            tc.tile_pool(name="psum", bufs=4, space="PSUM") as psum_pool,
            tc.tile_pool(name="warmps", bufs=1, space="PSUM") as warm_pool,
        ):
            # Constants via memset (no DMA dependency — early-phase DMA
            # completion latency is ~6us in this runtime).
            ones_t = const_pool.tile([1, B], _FP16)
            nc.vector.memset(ones_t, 1.0)
            warm_rhs = const_pool.tile([1, 512], _FP16)
            nc.vector.memset(warm_rhs, 1.0)

            # x_all gates every real matmul: issue it before the weight
            # stream hogs HBM. Scalar HWDGE ring; weights on the Sync ring.
            x_all = const_pool.tile([128, FPC, KT, B], _FP16)
            nc.scalar.dma_start(x_all, xt[:])
            b_all = const_pool.tile([1, FPC, C], _FP16)
            nc.scalar.dma_start(b_all, bs[:].rearrange("f o c -> o f c"))

            # Whole weight shard is SBUF-resident (8 x 16KB/partition): the
            # weight stream never stalls on buffer recycling, so the 16 MB
            # HBM read runs at full rate for the entire kernel. 1MB halves
            # let feature 0 start ~2.5us earlier and halve PE wait quanta.
            w_halves = []
            for f in range(FPC):
                hs = []
                for h in range(WH):
                    w_tile = wpool.tile([128, KH, C], _FP16, tag="w")
                    if f == FPC - 1 and h == WH - 1:
                        # Final piece in 256KB quarters: its completion sem
                        # gates the last feature, and receipt lag scales with
                        # how much data is queued behind the transfer.
                        for kk in range(KH):
                            nc.sync.dma_start(
                                w_tile[:, kk, :],
                                wt[f][:, h * KH + kk, :],
                            )
                    else:
                        nc.sync.dma_start(
                            w_tile, wt[f][:, h * KH : (h + 1) * KH, :]
                        )
                    hs.append(w_tile)
                w_halves.append(hs)

            # Keep the PE busy until W0h0 lands so HAM is warm for real work.
            warm_ps = warm_pool.tile([B, 512], _FP32)
            for _ in range(N_WARM):
                nc.tensor.matmul(warm_ps, ones_t, warm_rhs, start=True, stop=True)

            for f in range(FPC):
                o_tile = opool.tile([B, C], _FP16)
                # Interleave the two PSUM accumulation groups (separate
                # banks) k-major: only the k=KT-1 matmuls depend on the last
                # weight piece, so the PE tail after the final W byte is
                # ~4 matmuls instead of a full group.
                pss = [
                    psum_pool.tile([B, 512], _FP32, name=f"ps_{f}_{n}", tag="ps")
                    for n in range(NT)
                ]
                for k in range(KT - 1):
                    for n in range(NT):
                        nc.tensor.matmul(
                            pss[n],
                            x_all[:, f, k, :],
                            w_halves[f][k // KH][:, k % KH, n * 512 : (n + 1) * 512],
                            start=(k == 0),
                            stop=False,
                        )
                k = KT - 1
                for n in range(NT):
                    nc.tensor.matmul(
                        pss[n],
                        x_all[:, f, k, :],
                        w_halves[f][k // KH][:, k % KH, n * 512 : (n + 1) * 512],
                        start=False,
                        stop=False,
                    )
                    nc.tensor.matmul(
                        pss[n],
                        ones_t,
                        b_all[:, f, n * 512 : (n + 1) * 512],
                        start=False,
                        stop=True,
                    )
                    nc.vector.tensor_copy(o_tile[:, n * 512 : (n + 1) * 512], pss[n])
                    # SWDGE (gpsimd) path: separate DMASW sem lanes, so these
                    # compute-gated stores never block the HWDGE weight
                    # stream's lane rotation. The last feature's stores go on
                    # the (now idle) HWDGE ring to skip the serialized Q7
                    # issue path on the critical tail.
                    dma_eng = nc.scalar if f == FPC - 1 else nc.gpsimd
                    dma_eng.dma_start(
                        y[f][:, n * 512 : (n + 1) * 512],
                        o_tile[:, n * 512 : (n + 1) * 512],
                    )
    _split_sync_waits(nc)
    return nc


_NC = None


def _get_program():
    global _NC
    if _NC is None:
        _NC = _build_program()
    return _NC


def _prep_inputs(x, weight, bias):
    """Host-side packing into the per-core DMA-friendly layouts (fp16)."""
    x = np.asarray(x, dtype=np.float32).reshape(B, F, C)
    weight = np.asarray(weight, dtype=np.float32)
    bias = np.asarray(bias, dtype=np.float32)
    ones = np.ones((1, B), dtype=np.float16)
    in_maps = []
    for c in range(NCORES):
        f0 = c * FPC
        xs = x[:, f0 : f0 + FPC, :]  # [B, FPC, C]
        # xt[ct, f, k, b] = x[b, f0+f, k*128+ct]
        xt = np.ascontiguousarray(
            xs.reshape(B, FPC, KT, 128).transpose(3, 1, 2, 0).astype(np.float16)
        )
        ws = weight[f0 : f0 + FPC]  # [FPC, C(out), C(in)]
        # wt[f, ct, k, o] = W[f0+f, o, k*128+ct]
        wt = np.ascontiguousarray(
            ws.reshape(FPC, C, KT, 128).transpose(0, 3, 2, 1).astype(np.float16)
        )
        bsc = np.ascontiguousarray(
            bias[f0 : f0 + FPC, None, :].astype(np.float16)
        )
        in_maps.append({"xt": xt, "wt": wt, "bs": bsc, "ones": ones})
    return in_maps


LAST_EXEC_NS = None
TRACE = False


def kernel(x, weight, bias):
    global LAST_EXEC_NS
    from concourse.bass_utils import run_bass_kernel_spmd

    nc = _get_program()
    in_maps = _prep_inputs(x, weight, bias)
    core_ids = list(range(NCORES))
    kwargs = {}
    if TRACE:
        try:
            _install_ntff_hook()
            import concourse.bass_utils as _bu

            _bu.upload_artifacts = lambda tmpdir: tmpdir
            kwargs["trace"] = True
        except Exception:
            pass
    res = run_bass_kernel_spmd(nc, in_maps, core_ids, **kwargs)
    LAST_EXEC_NS = res.exec_time_ns
    ys = np.stack([res.results[c]["y"] for c in range(NCORES)])  # [NC, FPC, B, C]
    out = ys.astype(np.float32).transpose(2, 0, 1, 3).reshape(B, F, 32, 32)
    return np.ascontiguousarray(out)


def _install_ntff_hook():
    """run_bass_kernel_spmd(trace=True) under axon needs antenv.axon_hooks,
    absent from this image — synthesize it and register the ctypes hook."""
    import sys, types, importlib.util

    if "antenv.axon_hooks" in sys.modules:
        return
    mod = types.ModuleType("antenv.axon_hooks")
    _h = [None]
    mod.set_axon_ntff_profile_hook = lambda h: _h.__setitem__(0, h)
    mod.get_axon_ntff_profile_hook = lambda: _h[0]
    import antenv

    sys.modules["antenv.axon_hooks"] = mod
    antenv.axon_hooks = mod
    spec = importlib.util.spec_from_file_location(
        "_trn_boot_local", "/root/.axon_site/trn_agent_boot/trn_boot.py"
    )
    tb = importlib.util.module_from_spec(spec)
    spec.loader.exec_module(tb)
    hook = tb._ntff_profile_via_ctypes("/opt/axon/libaxon_pjrt.so")
    if hook is not None:
        mod.set_axon_ntff_profile_hook(hook)


# revision 20
# speedup vs baseline: 1.0005x; 1.0005x over previous
"""ChannelFC Trainium2 kernel: per-feature Linear y[b,f,:] = x[b,f,:] @ W[f].T + bias[f].

Shapes: x [64, 64, 32, 32], weight [64, 1024, 1024], bias [64, 1024].
Strategy: feature-parallel over 8 NeuronCores (8 features/core), fp16 operands
(fp32 PSUM accumulation), X-stationary matmuls (lhsT = X_f^T k-tiles [128, 64],
rhs = W_f^T k-tiles [128, 512]), bias added via a K=1 ones-vector matmul into
the same PSUM accumulation group. Weight stream is the roofline: 16 MB/core fp16.
"""

import numpy as np

import concourse.bass as bass
import concourse.mybir as mybir
from concourse.tile import TileContext
from concourse.vector_clock import ScopedClock


def _install_lean_tail_patch():
    """Tile's exit sequence is drain -> barrier -> sem-clear -> barrier
    (~7us measured). The final barrier only guards engines re-entering the
    sem space after the clear; at NEFF end nothing follows, and the next
    execution starts only after every engine's stream (including the
    GpSimd clear) has completed. Dropping it saves ~3-4us per run."""
    if getattr(TileContext, "_lean_tail", False):
        return

    def _drain_and_barrier(self, tick_clock, wait_clock):
        drain_inst = self.nc.sync.drain()
        wait_clock.add_sem_waits(
            drain_inst.ins, ScopedClock({None: tick_clock.global_clock})
        )
        self.nc.all_engine_barrier()
        assert self.sems is not None
        popped = self.nc._tile_sem_poison_stack.pop()
        assert popped is self._sem_poison
        self.nc.clear_and_free_semaphores(list(self.sems.allocated().values()))

    TileContext._drain_and_barrier = _drain_and_barrier
    TileContext._lean_tail = True


_install_lean_tail_patch()

B, F, C = 64, 64, 1024
NCORES = 8
FPC = F // NCORES  # features per core
KT = C // 128  # k-tiles of 128
NT = 2  # n-tiles of 512 (PSUM bank limit)
WH = 2  # W halves per feature (pipeline granularity)
KH = KT // WH  # k-tiles per half

_FP16 = mybir.dt.float16
_FP32 = mybir.dt.float32


def _split_sync_waits(nc, maxw=1):
    """This container's walrus build rejects more than one sync wait on an
    instruction ("Too many sync wait commands" in codegen). Hoist extra waits
    into same-engine NOPs placed immediately before the instruction —
    semantically identical since the engine sequencer blocks on each in order."""
    n = 0
    for fn in nc.m.functions:
        for bb in fn.blocks:
            new = []
            for inst in bb.instructions:
                si = getattr(inst, "sync_info", None)
                waits = list(si.on_wait or []) if si is not None else []
                if len(waits) > maxw:
                    extra, keep = waits[:-maxw], waits[-maxw:]
                    for i in range(0, len(extra), maxw):
                        n += 1
                        new.append(
                            mybir.InstNoOp(
                                name=f"WSPLIT-{n}",
                                engine=inst.engine,
                                bass_nofuse=True,
                                sync_info=mybir.SyncInfo(
                                    on_wait=extra[i : i + maxw], on_update=[]
                                ),
                            )
                        )
                    inst.sync_info = mybir.SyncInfo(
                        on_wait=keep, on_update=list(si.on_update or [])
                    )
                new.append(inst)
            bb.instructions = new


N_WARM = 22  # dummy K=1 N=512 matmuls bridging PE from t~8us to W0h0 (~13.5us)
# so the HAM clock gate is warm when real matmuls start.


def _build_program():
    nc = bass.Bass()
    xt = nc.dram_tensor("xt", [128, FPC, KT, B], _FP16, kind="ExternalInput")
    wt = nc.dram_tensor("wt", [FPC, 128, KT, C], _FP16, kind="ExternalInput")
    bs = nc.dram_tensor("bs", [FPC, 1, C], _FP16, kind="ExternalInput")
    y = nc.dram_tensor("y", [FPC, B, C], _FP16, kind="ExternalOutput")

    with TileContext(nc) as tc:
        with (
            tc.tile_pool(name="wpool", bufs=2 * FPC) as wpool,
            tc.tile_pool(name="const", bufs=1) as const_pool,
            tc.tile_pool(name="opool", bufs=3) as opool,
            tc.tile_pool(name="psum", bufs=4, space="PSUM") as psum_pool,
            tc.tile_pool(name="warmps", bufs=1, space="PSUM") as warm_pool,
        ):
            # Constants via memset (no DMA dependency — early-phase DMA
            # completion latency is ~6us in this runtime).
            ones_t = const_pool.tile([1, B], _FP16)
            nc.vector.memset(ones_t, 1.0)
            warm_rhs = const_pool.tile([1, 512], _FP16)
            nc.vector.memset(warm_rhs, 1.0)

            # x_all gates every real matmul: issue it before the weight
            # stream hogs HBM. Scalar HWDGE ring; weights on the Sync ring.
            x_all = const_pool.tile([128, FPC, KT, B], _FP16)
            nc.scalar.dma_start(x_all, xt[:])
            b_all = const_pool.tile([1, FPC, C], _FP16)
            nc.scalar.dma_start(b_all, bs[:].rearrange("f o c -> o f c"))

            # Whole weight shard is SBUF-resident (8 x 16KB/partition): the
            # weight stream never stalls on buffer recycling, so the 16 MB
            # HBM read runs at full rate for the entire kernel. 1MB halves
            # let feature 0 start ~2.5us earlier and halve PE wait quanta.
            w_halves = []
            for f in range(FPC):
                hs = []
                for h in range(WH):
                    w_tile = wpool.tile([128, KH, C], _FP16, tag="w")
                    if f == FPC - 1 and h == WH - 1:
                        # Final piece in 256KB quarters: its completion sem
                        # gates the last feature, and receipt lag scales with
                        # how much data is queued behind the transfer.
                        for kk in range(KH):
                            nc.sync.dma_start(
                                w_tile[:, kk, :],
                                wt[f][:, h * KH + kk, :],
                            )
                    else:
                        nc.sync.dma_start(
                            w_tile, wt[f][:, h * KH : (h + 1) * KH, :]
                        )
                    hs.append(w_tile)
                w_halves.append(hs)

            # Keep the PE busy until W0h0 lands so HAM is warm for real work.
            warm_ps = warm_pool.tile([B, 512], _FP32)
            for _ in range(N_WARM):
                nc.tensor.matmul(warm_ps, ones_t, warm_rhs, start=True, stop=True)

            for f in range(FPC):
                o_tile = opool.tile([B, C], _FP16)
                for n in range(NT):
                    ps = psum_pool.tile([B, 512], _FP32)
                    for k in range(KT):
                        nc.tensor.matmul(
                            ps,
                            x_all[:, f, k, :],
                            w_halves[f][k // KH][:, k % KH, n * 512 : (n + 1) * 512],
                            start=(k == 0),
                            stop=False,
                        )
                    nc.tensor.matmul(
                        ps,
                        ones_t,
                        b_all[:, f, n * 512 : (n + 1) * 512],
                        start=False,
                        stop=True,
                    )
                    nc.vector.tensor_copy(o_tile[:, n * 512 : (n + 1) * 512], ps)
                    # SWDGE (gpsimd) path: separate DMASW sem lanes, so these
                    # compute-gated stores never block the HWDGE weight
                    # stream's lane rotation. The last feature's stores go on
                    # the (now idle) HWDGE ring to skip the serialized Q7
                    # issue path on the critical tail.
                    dma_eng = nc.scalar if f == FPC - 1 else nc.gpsimd
                    dma_eng.dma_start(
                        y[f][:, n * 512 : (n + 1) * 512],
                        o_tile[:, n * 512 : (n + 1) * 512],
                    )
    _split_sync_waits(nc)
    return nc


_NC = None


def _get_program():
    global _NC
    if _NC is None:
        _NC = _build_program()
    return _NC


def _prep_inputs(x, weight, bias):
    """Host-side packing into the per-core DMA-friendly layouts (fp16)."""
    x = np.asarray(x, dtype=np.float32).reshape(B, F, C)
    weight = np.asarray(weight, dtype=np.float32)
    bias = np.asarray(bias, dtype=np.float32)
    ones = np.ones((1, B), dtype=np.float16)
    in_maps = []
    for c in range(NCORES):
        f0 = c * FPC
        xs = x[:, f0 : f0 + FPC, :]  # [B, FPC, C]
        # xt[ct, f, k, b] = x[b, f0+f, k*128+ct]
        xt = np.ascontiguousarray(
            xs.reshape(B, FPC, KT, 128).transpose(3, 1, 2, 0).astype(np.float16)
        )
        ws = weight[f0 : f0 + FPC]  # [FPC, C(out), C(in)]
        # wt[f, ct, k, o] = W[f0+f, o, k*128+ct]
        wt = np.ascontiguousarray(
            ws.reshape(FPC, C, KT, 128).transpose(0, 3, 2, 1).astype(np.float16)
        )
        bsc = np.ascontiguousarray(
            bias[f0 : f0 + FPC, None, :].astype(np.float16)
        )
        in_maps.append({"xt": xt, "wt": wt, "bs": bsc, "ones": ones})
    return in_maps


LAST_EXEC_NS = None
TRACE = False


def kernel(x, weight, bias):
    global LAST_EXEC_NS
    from concourse.bass_utils import run_bass_kernel_spmd

    nc = _get_program()
    in_maps = _prep_inputs(x, weight, bias)
    core_ids = list(range(NCORES))
    kwargs = {}
    if TRACE:
        try:
            _install_ntff_hook()
            import concourse.bass_utils as _bu

            _bu.upload_artifacts = lambda tmpdir: tmpdir
            kwargs["trace"] = True
        except Exception:
            pass
    res = run_bass_kernel_spmd(nc, in_maps, core_ids, **kwargs)
    LAST_EXEC_NS = res.exec_time_ns
    ys = np.stack([res.results[c]["y"] for c in range(NCORES)])  # [NC, FPC, B, C]
    out = ys.astype(np.float32).transpose(2, 0, 1, 3).reshape(B, F, 32, 32)
    return np.ascontiguousarray(out)


def _install_ntff_hook():
    """run_bass_kernel_spmd(trace=True) under axon needs antenv.axon_hooks,
    absent from this image — synthesize it and register the ctypes hook."""
    import sys, types, importlib.util

    if "antenv.axon_hooks" in sys.modules:
        return
    mod = types.ModuleType("antenv.axon_hooks")
    _h = [None]
    mod.set_axon_ntff_profile_hook = lambda h: _h.__setitem__(0, h)
    mod.get_axon_ntff_profile_hook = lambda: _h[0]
    import antenv

    sys.modules["antenv.axon_hooks"] = mod
    antenv.axon_hooks = mod
    spec = importlib.util.spec_from_file_location(
        "_trn_boot_local", "/root/.axon_site/trn_agent_boot/trn_boot.py"
    )
    tb = importlib.util.module_from_spec(spec)
    spec.loader.exec_module(tb)
    hook = tb._ntff_profile_via_ctypes("/opt/axon/libaxon_pjrt.so")
    if hook is not None:
        mod.set_axon_ntff_profile_hook(hook)
